# revision 22
# baseline (speedup 1.0000x reference)
"""Trainium2 Bass kernel for a PreNorm causal transformer block (dense_transformer).

Sharding (8 cores, zero-communication):
  core c -> batch b = c//2, "zigzag" half = c%2 of the 16 query tiles (128 tokens each).
  half 0 gets global query tiles [0,3,4,7,8,11,12,15]; half 1 gets [1,2,5,6,9,10,13,14].
  Local query tile j on either core needs only key tiles [0, 2j+2) -- a *uniform*
  bound, so one SPMD program serves both cores; causality inside the frontier is
  enforced by a host-provided additive mask.  Each core redundantly computes K/V
  for the full 2048-token batch (cheaper than any collective on this box).

  On-chip layout is feature-major ([D, tokens]) throughout, which makes every
  GEMM's output directly consumable by the next GEMM with zero transposes.
  Attention computes S^T ([keys, queries]); softmax runs max-free (scores are
  O(5), exp can't overflow) and the denominator falls out of the AV matmul via
  a ones-column appended to V.  All matmuls run in float32r (full PE rate,
  ~1e-4 relative rounding).
"""

import numpy as np

B, N, D = 4, 2048, 1024
H, DH = 16, 64
INNER = H * DH
FF = 4 * D
EPS = 1e-5
T = 1024           # local query tokens per core
NKT = 16           # key tiles of 128 in the full sequence
MASKVAL = -1e4     # additive; exp(0.125 * -1e4) underflows to 0.0

A_TILES = ([0, 3, 4, 7, 8, 11, 12, 15], [1, 2, 5, 6, 9, 10, 13, 14])

_PROG = None


def _build_program():
    import concourse.tile as tile
    from concourse import bacc, mybir
    from contextlib import ExitStack

    F32 = mybir.dt.float32
    F32R = mybir.dt.float32r
    AF = mybir.ActivationFunctionType
    ALU = mybir.AluOpType

    nc = bacc.Bacc("TRN2", target_bir_lowering=False, debug=False, num_devices=8)

    # ---- DRAM parameters (per-core shards prepared by the host) ----
    p_xq = nc.declare_dram_parameter("xq", [D, T], F32R, isOutput=False)       # feature-major local queries
    p_xkv = nc.declare_dram_parameter("xkv", [D, N], F32R, isOutput=False)     # feature-major full batch
    p_mask = nc.declare_dram_parameter("maskt", [NKT, 128, 128], F32, isOutput=False)
    p_wq = nc.declare_dram_parameter("wq", [8, 128, 8, 128], F32R, isOutput=False)
    p_wk = nc.declare_dram_parameter("wk", [8, 128, 8, 128], F32R, isOutput=False)
    p_wv = nc.declare_dram_parameter("wv", [8, 128, 8, 128], F32R, isOutput=False)
    p_wo = nc.declare_dram_parameter("wo", [8, 128, 8, 128], F32R, isOutput=False)
    p_w1 = nc.declare_dram_parameter("w1", [32, 128, 8, 128], F32R, isOutput=False)
    p_w2 = nc.declare_dram_parameter("w2", [8, 128, 32, 128], F32R, isOutput=False)
    p_ln1g = nc.declare_dram_parameter("ln1g", [128, 8], F32, isOutput=False)
    p_ln1b = nc.declare_dram_parameter("ln1b", [128, 8], F32, isOutput=False)
    p_ln2g = nc.declare_dram_parameter("ln2g", [128, 8], F32, isOutput=False)
    p_ln2b = nc.declare_dram_parameter("ln2b", [128, 8], F32, isOutput=False)
    p_bout = nc.declare_dram_parameter("bout", [128, 8], F32, isOutput=False)
    p_bff1 = nc.declare_dram_parameter("bff1", [128, 32], F32, isOutput=False)
    p_bff2 = nc.declare_dram_parameter("bff2", [128, 8], F32, isOutput=False)
    p_whalt = nc.declare_dram_parameter("whalt", [128, 8], F32, isOutput=False)

    p_xout = nc.declare_dram_parameter("xout", [D, T], F32, isOutput=True)     # feature-major final x
    p_halt = nc.declare_dram_parameter("hpart", [128, 1], F32, isOutput=True)

    d_xf2 = nc.dram_tensor("xf2_scratch", [D, T], F32R)                        # post-attention residual stream
    d_hq = nc.dram_tensor("hq_scratch", [D, T], F32R)                          # LN1(xq)

    with tile.TileContext(nc) as tc, ExitStack() as top:
        const = top.enter_context(tc.tile_pool(name="const", bufs=1))
        ones_f = const.tile([128, 1], F32, tag="onesf")
        nc.vector.memset(ones_f, 1.0)
        ones_r = const.tile([128, 1], F32R, tag="ones")
        nc.vector.tensor_copy(out=ones_r, in_=ones_f)
        eps_t = const.tile([1, 1], F32, tag="eps")
        nc.vector.memset(eps_t, EPS)
        vones_f = const.tile([128, NKT, 4, 1], F32, tag="vonesf")
        nc.vector.memset(vones_f, 1.0)

        def load_const(param, shape, tag):
            t = const.tile(shape, F32, tag=tag)
            nc.sync.dma_start(out=t, in_=param[:, :])
            return t

        ln1g = load_const(p_ln1g, [128, 8], "ln1g")
        ln1b = load_const(p_ln1b, [128, 8], "ln1b")
        ln2g = load_const(p_ln2g, [128, 8], "ln2g")
        ln2b = load_const(p_ln2b, [128, 8], "ln2b")
        bout = load_const(p_bout, [128, 8], "bout")
        bff1 = load_const(p_bff1, [128, 32], "bff1")
        bff2 = load_const(p_bff2, [128, 8], "bff2")
        whalt = load_const(p_whalt, [128, 8], "whalt")
        mask_sb = const.tile([128, NKT, 128], F32, tag="mask")
        nc.sync.dma_start(out=mask_sb, in_=p_mask.rearrange("t p q -> p t q"))

        def layernorm(ctx, x_dram, Ttok, g, b, out_tile=None, out_dram=None):
            """Feature-major LN: x [D, Ttok] in DRAM -> 128x8xTtok f32r (SBUF tile or DRAM).
            Stats via ones-matmul over partition chunks; x is streamed twice."""
            pool = ctx.enter_context(tc.tile_pool(name="lnp", bufs=2))
            stat = ctx.enter_context(tc.tile_pool(name="lnstat", bufs=2))
            psp = ctx.enter_context(tc.tile_pool(name="lnps", bufs=2, space="PSUM"))
            xv = x_dram.rearrange("(ko ki) t -> ki ko t", ki=128)
            ov = None
            if out_dram is not None:
                ov = out_dram.rearrange("(ko ki) t -> ki ko t", ki=128)
            for blk in range(Ttok // 512):
                c0 = blk * 512
                ps_s = psp.tile([1, 512], F32, tag="lnsum")
                ps_q = psp.tile([1, 512], F32, tag="lnsq")
                for k in range(8):
                    xt = pool.tile([128, 512], F32R, tag="ln_x")
                    nc.sync.dma_start(out=xt, in_=xv[:, k, c0:c0 + 512])
                    sq = pool.tile([128, 512], F32R, tag="ln_sq")
                    nc.vector.tensor_mul(out=sq, in0=xt, in1=xt)
                    nc.tensor.matmul(ps_s, ones_r, xt, start=(k == 0), stop=(k == 7))
                    nc.tensor.matmul(ps_q, ones_r, sq, start=(k == 0), stop=(k == 7))
                mu = stat.tile([1, 512], F32, tag="mu")
                nc.scalar.mul(out=mu, in_=ps_s, mul=1.0 / D)
                ex2 = stat.tile([1, 512], F32, tag="ex2")
                nc.scalar.mul(out=ex2, in_=ps_q, mul=1.0 / D)
                tmp = stat.tile([1, 512], F32, tag="tmp")
                nc.vector.tensor_mul(out=tmp, in0=mu, in1=mu)      # mu^2
                nc.vector.tensor_sub(out=ex2, in0=ex2, in1=tmp)    # var (in place)
                nc.scalar.activation(out=tmp, in_=ex2, func=AF.Sqrt, bias=eps_t, scale=1.0)
                rr = stat.tile([1, 512], F32R, tag="rr")
                m2 = stat.tile([1, 512], F32R, tag="m2")
                with nc.allow_low_precision(reason="f32r rounding is within kernel tolerance"):
                    nc.vector.reciprocal(out=rr, in_=tmp)          # r = rsqrt(var+eps)
                    nc.vector.tensor_mul(out=m2, in0=mu, in1=rr)   # mu*r
                # physically broadcast r and mu*r across partitions on gpsimd
                rb_t = pool.tile([128, 512], F32R, tag="ln_rb")
                nc.gpsimd.partition_broadcast(rb_t, rr)
                mb_t = pool.tile([128, 512], F32R, tag="ln_mb")
                nc.gpsimd.partition_broadcast(mb_t, m2)
                for k in range(8):
                    xt = pool.tile([128, 512], F32R, tag="ln_x2")
                    nc.sync.dma_start(out=xt, in_=xv[:, k, c0:c0 + 512])
                    t1 = pool.tile([128, 512], F32, tag="ln_t1")
                    nc.vector.tensor_mul(out=t1, in0=xt, in1=rb_t)
                    nc.vector.tensor_sub(out=t1, in0=t1, in1=mb_t)
                    if out_tile is not None:
                        nc.vector.tensor_scalar(out=out_tile[:, k, c0:c0 + 512], in0=t1,
                                                scalar1=g[:, k:k + 1], scalar2=b[:, k:k + 1],
                                                op0=ALU.mult, op1=ALU.add)
                    else:
                        ot = pool.tile([128, 512], F32R, tag="ln_o")
                        nc.vector.tensor_scalar(out=ot, in0=t1,
                                                scalar1=g[:, k:k + 1], scalar2=b[:, k:k + 1],
                                                op0=ALU.mult, op1=ALU.add)
                        nc.sync.dma_start(out=ov[:, k, c0:c0 + 512], in_=ot)

        with tc.tile_pool(name="bigp", bufs=1) as bigp:
            Hkv = bigp.tile([128, 8, N], F32R, tag="hkv")
            Of = bigp.tile([128, 8, T], F32R, tag="of")

            # ---- Phase 1: LayerNorm 1 (kv -> SBUF, q -> DRAM scratch) ----
            with ExitStack() as ph:
                layernorm(ph, p_xkv, N, ln1g, ln1b, out_tile=Hkv)
                layernorm(ph, p_xq, T, ln1g, ln1b, out_dram=d_hq.ap())

            with tc.tile_pool(name="qfp", bufs=1) as qfp:
                Qf = qfp.tile([128, 8, T], F32R, tag="qf")

                # ---- Phase 2: Q projection (weights stationary, feature-major out) ----
                hqv = d_hq.ap().rearrange("(ko ki) t -> ki ko t", ki=128)
                with ExitStack() as ph:
                    wpool = ph.enter_context(tc.tile_pool(name="wqp", bufs=2))
                    hpool = ph.enter_context(tc.tile_pool(name="hqtb", bufs=2))
                    qps = ph.enter_context(tc.tile_pool(name="qps", bufs=2, space="PSUM"))
                    for tb in range(2):
                        hq_tb = hpool.tile([128, 8, 512], F32R, tag="hqtb")
                        nc.sync.dma_start(out=hq_tb, in_=hqv[:, :, tb * 512:(tb + 1) * 512])
                        for p in range(8):
                            wq_p = wpool.tile([128, 8, 128], F32R, tag="wq")
                            nc.sync.dma_start(out=wq_p, in_=p_wq[p])
                            ps = qps.tile([128, 512], F32, tag="qproj")
                            for k in range(8):
                                nc.tensor.matmul(ps, wq_p[:, k, :], hq_tb[:, k, :],
                                                 start=(k == 0), stop=(k == 7))
                            nc.vector.tensor_copy(out=Qf[:, p, tb * 512:(tb + 1) * 512], in_=ps)

                # ---- Phase 3: per-group V projection + per-pair K projection & attention ----
                with ExitStack() as ph:
                    vpool = ph.enter_context(tc.tile_pool(name="vsbp", bufs=1))
                    kpool = ph.enter_context(tc.tile_pool(name="kfp", bufs=1))
                    wkv = ph.enter_context(tc.tile_pool(name="wkvp", bufs=1))
                    ptp = ph.enter_context(tc.tile_pool(name="ptp", bufs=1))
                    recp = ph.enter_context(tc.tile_pool(name="recp", bufs=2))
                    pps = ph.enter_context(tc.tile_pool(name="pps", bufs=2, space="PSUM"))
                    sps = ph.enter_context(tc.tile_pool(name="sps", bufs=2, space="PSUM"))
                    ops = ph.enter_context(tc.tile_pool(name="opsp", bufs=1, space="PSUM"))
                    for g in range(4):
                        # V for heads 4g..4g+3, keys-major, with a ones column per head
                        wv_g = wkv.tile([128, 8, 256], F32R, tag="wv")
                        nc.sync.dma_start(out=wv_g.rearrange("p k (c m) -> p k c m", c=2),
                                          in_=p_wv.rearrange("c p k m -> p k c m")[:, :, 2 * g:2 * g + 2, :])
                        vsb = vpool.tile([128, NKT, 4, 65], F32R, tag="vsb")
                        nc.vector.tensor_copy(out=vsb[:, :, :, 64:65], in_=vones_f)
                        for t16 in range(NKT):
                            psv = pps.tile([128, 256], F32, tag="proj")
                            for k in range(8):
                                nc.tensor.matmul(psv, Hkv[:, k, t16 * 128:(t16 + 1) * 128], wv_g[:, k, :],
                                                 start=(k == 0), stop=(k == 7))
                            nc.vector.tensor_copy(out=vsb[:, t16, :, 0:64],
                                                  in_=psv.rearrange("p (h d) -> p h d", d=64))
                        for p in (2 * g, 2 * g + 1):
                            wk_p = wkv.tile([128, 8, 128], F32R, tag="wk")
                            nc.sync.dma_start(out=wk_p, in_=p_wk[p])
                            kf = kpool.tile([128, N], F32R, tag="kf")
                            for tb in range(4):
                                psk = pps.tile([128, 512], F32, tag="proj")
                                for k in range(8):
                                    nc.tensor.matmul(psk, wk_p[:, k, :], Hkv[:, k, tb * 512:(tb + 1) * 512],
                                                     start=(k == 0), stop=(k == 7))
                                nc.vector.tensor_copy(out=kf[:, tb * 512:(tb + 1) * 512], in_=psk)
                            # attention for heads 2p (h01=0) and 2p+1 (h01=1)
                            for qh in range(2):
                                kts = range(8) if qh == 0 else range(16)
                                last_kt = 7 if qh == 0 else 15
                                o_ps0 = ops.tile([65, 512], F32, tag="ops0")
                                o_ps1 = ops.tile([65, 512], F32, tag="ops1")
                                o_ps = [o_ps0, o_ps1]
                                for kt in kts:
                                    q0 = 128 * (kt // 2)
                                    lq = max(q0 - 512 * qh, 0)
                                    gq0 = qh * 512
                                    for h01 in range(2):
                                        hh = 64 * h01
                                        s_ps = sps.tile([128, 512], F32, tag=f"sps{h01}")
                                        nc.tensor.matmul(s_ps[:, lq:512],
                                                         kf[hh:hh + 64, kt * 128:(kt + 1) * 128],
                                                         Qf[hh:hh + 64, p, gq0 + lq:gq0 + 512],
                                                         start=True, stop=True)
                                        if qh == 0 or kt >= 8:
                                            # additive causal/frontier mask on the first query block
                                            nc.vector.tensor_add(out=s_ps[:, lq:lq + 128],
                                                                 in0=s_ps[:, lq:lq + 128],
                                                                 in1=mask_sb[:, kt, :])
                                        pt = ptp.tile([128, 512], F32R, tag=f"pt{h01}")
                                        nc.scalar.activation(out=pt[:, lq:512], in_=s_ps[:, lq:512],
                                                             func=AF.Exp, scale=0.125)
                                        nc.tensor.matmul(o_ps[h01][:, lq:512],
                                                         vsb[:, kt, 2 * (p - 2 * g) + h01, :],
                                                         pt[:, lq:512],
                                                         start=(kt == 0), stop=(kt == last_kt))
                                for h01 in range(2):
                                    hh = 64 * h01
                                    rec = recp.tile([1, 512], F32, tag="rec")
                                    nc.vector.reciprocal(out=rec, in_=o_ps[h01][64:65, :])
                                    rec_b = recp.tile([64, 512], F32, tag="recb")
                                    nc.gpsimd.partition_broadcast(rec_b, rec)
                                    nc.vector.tensor_mul(
                                        out=Of[hh:hh + 64, p, qh * 512:(qh + 1) * 512],
                                        in0=o_ps[h01][0:64, :],
                                        in1=rec_b)

            # ---- Phase 4: output projection + residual -> xf2 (DRAM scratch) ----
            xf2v = d_xf2.ap().rearrange("(ko ki) t -> ki ko t", ki=128)
            xqv = p_xq.rearrange("(ko ki) t -> ki ko t", ki=128)
            with ExitStack() as ph:
                wpool = ph.enter_context(tc.tile_pool(name="wop", bufs=2))
                spool = ph.enter_context(tc.tile_pool(name="osb", bufs=3))
                qps = ph.enter_context(tc.tile_pool(name="ops2", bufs=2, space="PSUM"))
                for of in range(8):
                    wo_t = wpool.tile([128, 8, 128], F32R, tag="wo")
                    nc.sync.dma_start(out=wo_t, in_=p_wo[of])
                    for tb in range(2):
                        ps = qps.tile([128, 512], F32, tag="oproj")
                        for k in range(8):
                            nc.tensor.matmul(ps, wo_t[:, k, :], Of[:, k, tb * 512:(tb + 1) * 512],
                                             start=(k == 0), stop=(k == 7))
                        xq_t = spool.tile([128, 512], F32R, tag="xqres")
                        nc.sync.dma_start(out=xq_t, in_=xqv[:, of, tb * 512:(tb + 1) * 512])
                        x2 = spool.tile([128, 512], F32R, tag="xf2t")
                        nc.vector.scalar_tensor_tensor(out=x2, in0=ps, scalar=bout[:, of:of + 1],
                                                       in1=xq_t, op0=ALU.add, op1=ALU.add)
                        nc.sync.dma_start(out=xf2v[:, of, tb * 512:(tb + 1) * 512], in_=x2)

        # ---- Phase 5: LayerNorm 2 (from scratch DRAM) ----
        with tc.tile_pool(name="h2p", bufs=1) as h2p:
            H2 = h2p.tile([128, 8, T], F32R, tag="h2")
            with ExitStack() as ph:
                layernorm(ph, d_xf2.ap(), T, ln2g, ln2b, out_tile=H2)

            # ---- Phase 6: FFN + residual + halt ----
            xf2v = d_xf2.ap().rearrange("(ko ki) t -> ki ko t", ki=128)
            hacc = const.tile([128, 8], F32, tag="hacc")
            with ExitStack() as ph:
                wpool = ph.enter_context(tc.tile_pool(name="wffp", bufs=2))
                gpool = ph.enter_context(tc.tile_pool(name="gffp", bufs=1))
                spool = ph.enter_context(tc.tile_pool(name="ffsb", bufs=3))
                fps = ph.enter_context(tc.tile_pool(name="ffps", bufs=2, space="PSUM"))
                for th in range(2):
                    c0 = th * 512
                    gts = []
                    for fb in range(32):
                        w1_t = wpool.tile([128, 8, 128], F32R, tag="w1")
                        nc.sync.dma_start(out=w1_t, in_=p_w1[fb])
                        ps1 = fps.tile([128, 512], F32, tag="ff1")
                        for k in range(8):
                            nc.tensor.matmul(ps1, w1_t[:, k, :], H2[:, k, c0:c0 + 512],
                                             start=(k == 0), stop=(k == 7))
                        g_t = gpool.tile([128, 512], F32R, tag=f"g{fb}")
                        nc.scalar.activation(out=g_t, in_=ps1, func=AF.Gelu,
                                             bias=bff1[:, fb:fb + 1], scale=1.0)
                        gts.append(g_t)
                    for of in range(8):
                        w2_t = wpool.tile([128, 32, 128], F32R, tag="w2")
                        nc.sync.dma_start(out=w2_t, in_=p_w2[of])
                        ps2 = fps.tile([128, 512], F32, tag="ff2")
                        for fb in range(32):
                            nc.tensor.matmul(ps2, w2_t[:, fb, :], gts[fb],
                                             start=(fb == 0), stop=(fb == 31))
                        xf2s = spool.tile([128, 512], F32R, tag="xf2res")
                        nc.sync.dma_start(out=xf2s, in_=xf2v[:, of, c0:c0 + 512])
                        xo = spool.tile([128, 512], F32, tag="xout")
                        nc.vector.scalar_tensor_tensor(out=xo, in0=ps2, scalar=bff2[:, of:of + 1],
                                                       in1=xf2s, op0=ALU.add, op1=ALU.add)
                        nc.sync.dma_start(out=p_xout[of * 128:(of + 1) * 128, c0:c0 + 512], in_=xo)
                        # halt partial: accumulate token-sums of the final x
                        rt = spool.tile([128, 1], F32, tag="hred")
                        nc.vector.reduce_sum(out=rt, in_=xo, axis=mybir.AxisListType.X)
                        if th == 0:
                            nc.vector.tensor_copy(out=hacc[:, of:of + 1], in_=rt)
                        else:
                            nc.vector.tensor_add(out=hacc[:, of:of + 1], in0=hacc[:, of:of + 1], in1=rt)
                hw_t = const.tile([128, 8], F32, tag="hw")
                nc.vector.tensor_mul(out=hw_t, in0=hacc, in1=whalt)
                hredf = const.tile([128, 1], F32, tag="hredf")
                nc.vector.reduce_sum(out=hredf, in_=hw_t, axis=mybir.AxisListType.X)
                nc.sync.dma_start(out=p_halt[:, :], in_=hredf)

    nc.finalize()
    return nc


def _pack_w(w, ncb):
    """[Din, Dout] -> [Dout/128, 128, Din/128, 128] with [cb, ki, ko, m] = w[ko*128+ki, cb*128+m]."""
    din, dout = w.shape
    return np.ascontiguousarray(
        w.reshape(din // 128, 128, dout // 128, 128).transpose(2, 1, 0, 3))


def _make_masks(a_tiles):
    """Additive S^T masks, [NKT, 128 keys, 128 queries], for the first query block of each kt strip."""
    m = np.zeros((NKT, 128, 128), np.float32)
    kk = np.arange(128)[:, None]
    qq = np.arange(128)[None, :]
    for kt in range(NKT):
        aj = a_tiles[kt // 2]
        if aj == kt:
            m[kt] = np.where(kk <= qq, 0.0, MASKVAL)
        elif aj < kt:
            m[kt] = MASKVAL
    return m


def kernel(**inputs):
    global _PROG
    from concourse.bass_utils import run_bass_kernel_spmd

    if _PROG is None:
        _PROG = _build_program()
    nc = _PROG

    x = np.asarray(inputs["x"], np.float32)
    w_qkv = np.asarray(inputs["w_qkv"], np.float32)
    wq, wk, wv = w_qkv[:, :INNER], w_qkv[:, INNER:2 * INNER], w_qkv[:, 2 * INNER:]
    shared = {
        "wq": _pack_w(wq, 8), "wk": _pack_w(wk, 8), "wv": _pack_w(wv, 8),
        "wo": _pack_w(np.asarray(inputs["w_out"], np.float32), 8),
        "w1": _pack_w(np.asarray(inputs["w_ff1"], np.float32), 32),
        "w2": _pack_w(np.asarray(inputs["w_ff2"], np.float32), 8),
        "ln1g": np.ascontiguousarray(np.asarray(inputs["ln1_g"], np.float32).reshape(8, 128).T),
        "ln1b": np.ascontiguousarray(np.asarray(inputs["ln1_b"], np.float32).reshape(8, 128).T),
        "ln2g": np.ascontiguousarray(np.asarray(inputs["ln2_g"], np.float32).reshape(8, 128).T),
        "ln2b": np.ascontiguousarray(np.asarray(inputs["ln2_b"], np.float32).reshape(8, 128).T),
        "bout": np.ascontiguousarray(np.asarray(inputs["b_out"], np.float32).reshape(8, 128).T),
        "bff1": np.ascontiguousarray(np.asarray(inputs["b_ff1"], np.float32).reshape(32, 128).T),
        "bff2": np.ascontiguousarray(np.asarray(inputs["b_ff2"], np.float32).reshape(8, 128).T),
        "whalt": np.ascontiguousarray(np.asarray(inputs["w_halt"], np.float32).reshape(8, 128).T),
    }
    masks = [_make_masks(A_TILES[0]), _make_masks(A_TILES[1])]

    in_maps = []
    for c in range(8):
        b, half = c // 2, c % 2
        tiles = A_TILES[half]
        xb = x[b]                                  # [N, D]
        xq = np.concatenate([xb[t * 128:(t + 1) * 128] for t in tiles], axis=0)  # [T, D]
        in_maps.append({
            "xq": np.ascontiguousarray(xq.T),
            "xkv": np.ascontiguousarray(xb.T),
            "maskt": masks[half],
            **shared,
        })

    res = run_bass_kernel_spmd(nc, in_maps, list(range(8)))
    globals()["LAST_RESULT"] = res

    x_out = np.empty((B, N, D), np.float32)
    hp = np.empty(8, np.float64)
    for c in range(8):
        b, half = c // 2, c % 2
        tiles = A_TILES[half]
        xo = res.results[c]["xout"].T              # [T tokens, D]
        for j, t in enumerate(tiles):
            x_out[b, t * 128:(t + 1) * 128] = xo[j * 128:(j + 1) * 128]
        hp[c] = res.results[c]["hpart"].sum()
    b_halt = np.asarray(inputs["b_halt"], np.float32)
    halt = ((hp[0::2] + hp[1::2]) / N + b_halt[0]).astype(np.float32)
    return x_out, halt


# revision 24
# speedup vs baseline: 1.6904x; 1.6904x over previous
"""Trainium2 Bass kernel for a PreNorm causal transformer block (dense_transformer).

Sharding (8 cores, zero-communication):
  core c -> batch b = c//2, "zigzag" half = c%2 of the 16 query tiles (128 tokens each).
  half 0 gets global query tiles [0,3,4,7,8,11,12,15]; half 1 gets [1,2,5,6,9,10,13,14].
  Local query tile j on either core needs only key tiles [0, 2j+2) -- a *uniform*
  bound, so one SPMD program serves both cores; causality inside the frontier is
  enforced by a host-provided additive mask.  Each core redundantly computes K/V
  for the full 2048-token batch (cheaper than any collective on this box).

  On-chip layout is feature-major ([D, tokens]) throughout, which makes every
  GEMM's output directly consumable by the next GEMM with zero transposes.
  Attention computes S^T ([keys, queries]); softmax runs max-free (scores are
  O(5), exp can't overflow) and the denominator falls out of the AV matmul via
  a ones-column appended to V.  All matmuls run in float32r (full PE rate,
  ~1e-4 relative rounding).
"""

import numpy as np

B, N, D = 4, 2048, 1024
H, DH = 16, 64
INNER = H * DH
FF = 4 * D
EPS = 1e-5
T = 1024           # local query tokens per core
NKT = 16           # key tiles of 128 in the full sequence
MASKVAL = -1e4     # additive; exp(0.125 * -1e4) underflows to 0.0

A_TILES = ([0, 3, 4, 7, 8, 11, 12, 15], [1, 2, 5, 6, 9, 10, 13, 14])

_PROG = None
LAST_RESULT = None
LAST_IN_MAPS = None


def _build_program():
    import concourse.tile as tile
    from concourse import bacc, mybir
    from contextlib import ExitStack

    F32 = mybir.dt.float32
    F32R = mybir.dt.float32r
    AF = mybir.ActivationFunctionType
    ALU = mybir.AluOpType

    nc = bacc.Bacc("TRN2", target_bir_lowering=False, debug=False, num_devices=8)

    # ---- DRAM parameters (per-core shards prepared by the host) ----
    p_xq = nc.declare_dram_parameter("xq", [D, T], F32R, isOutput=False)       # feature-major local queries
    p_xkv = nc.declare_dram_parameter("xkv", [D, N], F32R, isOutput=False)     # feature-major full batch
    p_mask = nc.declare_dram_parameter("maskt", [NKT, 128, 128], F32, isOutput=False)
    p_wq = nc.declare_dram_parameter("wq", [8, 128, 8, 128], F32R, isOutput=False)
    p_wk = nc.declare_dram_parameter("wk", [8, 128, 8, 128], F32R, isOutput=False)
    p_wv = nc.declare_dram_parameter("wv", [8, 128, 8, 128], F32R, isOutput=False)
    p_wo = nc.declare_dram_parameter("wo", [8, 128, 8, 128], F32R, isOutput=False)
    p_w1 = nc.declare_dram_parameter("w1", [32, 128, 8, 128], F32R, isOutput=False)
    p_w2 = nc.declare_dram_parameter("w2", [8, 128, 32, 128], F32R, isOutput=False)
    p_ln1g = nc.declare_dram_parameter("ln1g", [128, 8], F32, isOutput=False)
    p_ln1b = nc.declare_dram_parameter("ln1b", [128, 8], F32, isOutput=False)
    p_ln2g = nc.declare_dram_parameter("ln2g", [128, 8], F32, isOutput=False)
    p_ln2b = nc.declare_dram_parameter("ln2b", [128, 8], F32, isOutput=False)
    p_bout = nc.declare_dram_parameter("bout", [128, 8], F32, isOutput=False)
    p_bff1 = nc.declare_dram_parameter("bff1", [128, 32], F32, isOutput=False)
    p_bff2 = nc.declare_dram_parameter("bff2", [128, 8], F32, isOutput=False)
    p_whalt = nc.declare_dram_parameter("whalt", [128, 8], F32, isOutput=False)

    p_xout = nc.declare_dram_parameter("xout", [D, T], F32, isOutput=True)     # feature-major final x
    p_halt = nc.declare_dram_parameter("hpart", [128, 1], F32, isOutput=True)

    d_xf2 = nc.dram_tensor("xf2_scratch", [D, T], F32R)                        # post-attention residual stream
    d_hq = nc.dram_tensor("hq_scratch", [D, T], F32R)                          # LN1(xq)

    with tile.TileContext(nc) as tc, ExitStack() as top:
        const = top.enter_context(tc.tile_pool(name="const", bufs=1))
        ones_f = const.tile([128, 1], F32, tag="onesf")
        nc.vector.memset(ones_f, 1.0)
        ones_r = const.tile([128, 1], F32R, tag="ones")
        nc.vector.tensor_copy(out=ones_r, in_=ones_f)
        eps_t = const.tile([1, 1], F32, tag="eps")
        nc.vector.memset(eps_t, EPS)
        vones_f = const.tile([128, NKT, 4, 1], F32, tag="vonesf")
        nc.vector.memset(vones_f, 1.0)

        def load_const(param, shape, tag):
            t = const.tile(shape, F32, tag=tag)
            nc.sync.dma_start(out=t, in_=param[:, :])
            return t

        ln1g = load_const(p_ln1g, [128, 8], "ln1g")
        ln1b = load_const(p_ln1b, [128, 8], "ln1b")
        ln2g = load_const(p_ln2g, [128, 8], "ln2g")
        ln2b = load_const(p_ln2b, [128, 8], "ln2b")
        bout = load_const(p_bout, [128, 8], "bout")
        bff1 = load_const(p_bff1, [128, 32], "bff1")
        bff2 = load_const(p_bff2, [128, 8], "bff2")
        whalt = load_const(p_whalt, [128, 8], "whalt")
        mask_sb = const.tile([128, NKT, 128], F32, tag="mask")
        nc.sync.dma_start(out=mask_sb, in_=p_mask.rearrange("t p q -> p t q"))

        def layernorm(ctx, x_dram, Ttok, g, b, out_tile=None, out_dram=None):
            """Feature-major LN: x [D, Ttok] in DRAM -> 128x8xTtok f32r (SBUF tile or DRAM).
            Stats via ones-matmul over partition chunks; x is streamed twice."""
            pool = ctx.enter_context(tc.tile_pool(name="lnp", bufs=2))
            stat = ctx.enter_context(tc.tile_pool(name="lnstat", bufs=2))
            psp = ctx.enter_context(tc.tile_pool(name="lnps", bufs=2, space="PSUM"))
            xv = x_dram.rearrange("(ko ki) t -> ki ko t", ki=128)
            ov = None
            if out_dram is not None:
                ov = out_dram.rearrange("(ko ki) t -> ki ko t", ki=128)
            for blk in range(Ttok // 512):
                c0 = blk * 512
                ps_s = psp.tile([1, 512], F32, tag="lnsum")
                ps_q = psp.tile([1, 512], F32, tag="lnsq")
                for k in range(8):
                    xt = pool.tile([128, 512], F32R, tag="ln_x")
                    nc.sync.dma_start(out=xt, in_=xv[:, k, c0:c0 + 512])
                    sq = pool.tile([128, 512], F32R, tag="ln_sq")
                    nc.vector.tensor_mul(out=sq, in0=xt, in1=xt)
                    nc.tensor.matmul(ps_s, ones_r, xt, start=(k == 0), stop=(k == 7))
                    nc.tensor.matmul(ps_q, ones_r, sq, start=(k == 0), stop=(k == 7))
                mu = stat.tile([1, 512], F32, tag="mu")
                nc.scalar.mul(out=mu, in_=ps_s, mul=1.0 / D)
                ex2 = stat.tile([1, 512], F32, tag="ex2")
                nc.scalar.mul(out=ex2, in_=ps_q, mul=1.0 / D)
                tmp = stat.tile([1, 512], F32, tag="tmp")
                nc.vector.tensor_mul(out=tmp, in0=mu, in1=mu)      # mu^2
                nc.vector.tensor_sub(out=ex2, in0=ex2, in1=tmp)    # var (in place)
                nc.scalar.activation(out=tmp, in_=ex2, func=AF.Sqrt, bias=eps_t, scale=1.0)
                rr = stat.tile([1, 512], F32R, tag="rr")
                m2 = stat.tile([1, 512], F32R, tag="m2")
                with nc.allow_low_precision(reason="f32r rounding is within kernel tolerance"):
                    nc.vector.reciprocal(out=rr, in_=tmp)          # r = rsqrt(var+eps)
                    nc.vector.tensor_mul(out=m2, in0=mu, in1=rr)   # mu*r
                # physically broadcast r and mu*r across partitions on gpsimd
                rb_t = pool.tile([128, 512], F32R, tag="ln_rb")
                nc.gpsimd.partition_broadcast(rb_t, rr)
                mb_t = pool.tile([128, 512], F32R, tag="ln_mb")
                nc.gpsimd.partition_broadcast(mb_t, m2)
                for k in range(8):
                    xt = pool.tile([128, 512], F32R, tag="ln_x2")
                    nc.sync.dma_start(out=xt, in_=xv[:, k, c0:c0 + 512])
                    t1 = pool.tile([128, 512], F32, tag="ln_t1")
                    nc.vector.tensor_mul(out=t1, in0=xt, in1=rb_t)
                    nc.vector.tensor_sub(out=t1, in0=t1, in1=mb_t)
                    if out_tile is not None:
                        nc.vector.tensor_scalar(out=out_tile[:, k, c0:c0 + 512], in0=t1,
                                                scalar1=g[:, k:k + 1], scalar2=b[:, k:k + 1],
                                                op0=ALU.mult, op1=ALU.add)
                    else:
                        ot = pool.tile([128, 512], F32R, tag="ln_o")
                        nc.vector.tensor_scalar(out=ot, in0=t1,
                                                scalar1=g[:, k:k + 1], scalar2=b[:, k:k + 1],
                                                op0=ALU.mult, op1=ALU.add)
                        nc.sync.dma_start(out=ov[:, k, c0:c0 + 512], in_=ot)

        with tc.tile_pool(name="bigp", bufs=1) as bigp:
            Hkv = bigp.tile([128, 8, N], F32R, tag="hkv")
            Of = bigp.tile([128, 8, T], F32R, tag="of")

            # ---- Phase 1: LayerNorm 1 (kv -> SBUF, q -> DRAM scratch) ----
            with ExitStack() as ph:
                layernorm(ph, p_xkv, N, ln1g, ln1b, out_tile=Hkv)
                layernorm(ph, p_xq, T, ln1g, ln1b, out_dram=d_hq.ap())

            with tc.tile_pool(name="qfp", bufs=1) as qfp:
                Qf = qfp.tile([128, 8, T], F32R, tag="qf")

                # ---- Phase 2: Q projection (weights stationary, feature-major out) ----
                hqv = d_hq.ap().rearrange("(ko ki) t -> ki ko t", ki=128)
                with ExitStack() as ph:
                    wpool = ph.enter_context(tc.tile_pool(name="wqp", bufs=2))
                    hpool = ph.enter_context(tc.tile_pool(name="hqtb", bufs=2))
                    qps = ph.enter_context(tc.tile_pool(name="qps", bufs=2, space="PSUM"))
                    for tb in range(2):
                        hq_tb = hpool.tile([128, 8, 512], F32R, tag="hqtb")
                        nc.sync.dma_start(out=hq_tb, in_=hqv[:, :, tb * 512:(tb + 1) * 512])
                        for p in range(8):
                            wq_p = wpool.tile([128, 8, 128], F32R, tag="wq")
                            nc.sync.dma_start(out=wq_p, in_=p_wq[p])
                            ps = qps.tile([128, 512], F32, tag="qproj")
                            for k in range(8):
                                nc.tensor.matmul(ps, wq_p[:, k, :], hq_tb[:, k, :],
                                                 start=(k == 0), stop=(k == 7))
                            nc.vector.tensor_copy(out=Qf[:, p, tb * 512:(tb + 1) * 512], in_=ps)

                # ---- Phase 3: per-group V projection + per-pair K projection & attention ----
                with ExitStack() as ph:
                    vpool = ph.enter_context(tc.tile_pool(name="vsbp", bufs=1))
                    kpool = ph.enter_context(tc.tile_pool(name="kfp", bufs=1))
                    wkv = ph.enter_context(tc.tile_pool(name="wkvp", bufs=1))
                    ptp = ph.enter_context(tc.tile_pool(name="ptp", bufs=1))
                    recp = ph.enter_context(tc.tile_pool(name="recp", bufs=2))
                    pps = ph.enter_context(tc.tile_pool(name="pps", bufs=2, space="PSUM"))
                    sps = ph.enter_context(tc.tile_pool(name="sps", bufs=2, space="PSUM"))
                    ops = ph.enter_context(tc.tile_pool(name="opsp", bufs=1, space="PSUM"))
                    for g in range(4):
                        # V for heads 4g..4g+3, keys-major, with a ones column per head
                        wv_g = wkv.tile([128, 8, 256], F32R, tag="wv")
                        nc.sync.dma_start(out=wv_g.rearrange("p k (c m) -> p k c m", c=2),
                                          in_=p_wv.rearrange("c p k m -> p k c m")[:, :, 2 * g:2 * g + 2, :])
                        vsb = vpool.tile([128, NKT, 4, 65], F32R, tag="vsb")
                        nc.vector.tensor_copy(out=vsb[:, :, :, 64:65], in_=vones_f)
                        for t16 in range(NKT):
                            psv = pps.tile([128, 256], F32, tag="proj")
                            for k in range(8):
                                nc.tensor.matmul(psv, Hkv[:, k, t16 * 128:(t16 + 1) * 128], wv_g[:, k, :],
                                                 start=(k == 0), stop=(k == 7))
                            nc.vector.tensor_copy(out=vsb[:, t16, :, 0:64],
                                                  in_=psv.rearrange("p (h d) -> p h d", d=64))
                        for p in (2 * g, 2 * g + 1):
                            wk_p = wkv.tile([128, 8, 128], F32R, tag="wk")
                            nc.sync.dma_start(out=wk_p, in_=p_wk[p])
                            kf = kpool.tile([128, N], F32R, tag="kf")
                            for tb in range(4):
                                psk = pps.tile([128, 512], F32, tag="proj")
                                for k in range(8):
                                    nc.tensor.matmul(psk, wk_p[:, k, :], Hkv[:, k, tb * 512:(tb + 1) * 512],
                                                     start=(k == 0), stop=(k == 7))
                                nc.vector.tensor_copy(out=kf[:, tb * 512:(tb + 1) * 512], in_=psk)
                            # attention for heads 2p (h01=0) and 2p+1 (h01=1)
                            for qh in range(2):
                                kts = range(8) if qh == 0 else range(16)
                                last_kt = 7 if qh == 0 else 15
                                o_ps0 = ops.tile([65, 512], F32, tag="ops0")
                                o_ps1 = ops.tile([65, 512], F32, tag="ops1")
                                o_ps = [o_ps0, o_ps1]
                                for kt in kts:
                                    q0 = 128 * (kt // 2)
                                    lq = max(q0 - 512 * qh, 0)
                                    gq0 = qh * 512
                                    for h01 in range(2):
                                        hh = 64 * h01
                                        s_ps = sps.tile([128, 512], F32, tag=f"sps{h01}")
                                        nc.tensor.matmul(s_ps[:, lq:512],
                                                         kf[hh:hh + 64, kt * 128:(kt + 1) * 128],
                                                         Qf[hh:hh + 64, p, gq0 + lq:gq0 + 512],
                                                         start=True, stop=True)
                                        if qh == 0 or kt >= 8:
                                            # additive causal/frontier mask on the first query block
                                            nc.vector.tensor_add(out=s_ps[:, lq:lq + 128],
                                                                 in0=s_ps[:, lq:lq + 128],
                                                                 in1=mask_sb[:, kt, :])
                                        pt = ptp.tile([128, 512], F32R, tag=f"pt{h01}")
                                        nc.scalar.activation(out=pt[:, lq:512], in_=s_ps[:, lq:512],
                                                             func=AF.Exp, scale=0.125)
                                        nc.tensor.matmul(o_ps[h01][:, lq:512],
                                                         vsb[:, kt, 2 * (p - 2 * g) + h01, :],
                                                         pt[:, lq:512],
                                                         start=(kt == 0), stop=(kt == last_kt))
                                for h01 in range(2):
                                    hh = 64 * h01
                                    rec = recp.tile([1, 512], F32, tag="rec")
                                    nc.vector.reciprocal(out=rec, in_=o_ps[h01][64:65, :])
                                    rec_b = recp.tile([64, 512], F32, tag="recb")
                                    nc.gpsimd.partition_broadcast(rec_b, rec)
                                    nc.vector.tensor_mul(
                                        out=Of[hh:hh + 64, p, qh * 512:(qh + 1) * 512],
                                        in0=o_ps[h01][0:64, :],
                                        in1=rec_b)

            # ---- Phase 4: output projection + residual -> xf2 (DRAM scratch) ----
            xf2v = d_xf2.ap().rearrange("(ko ki) t -> ki ko t", ki=128)
            xqv = p_xq.rearrange("(ko ki) t -> ki ko t", ki=128)
            with ExitStack() as ph:
                wpool = ph.enter_context(tc.tile_pool(name="wop", bufs=2))
                spool = ph.enter_context(tc.tile_pool(name="osb", bufs=3))
                qps = ph.enter_context(tc.tile_pool(name="ops2", bufs=2, space="PSUM"))
                for of in range(8):
                    wo_t = wpool.tile([128, 8, 128], F32R, tag="wo")
                    nc.sync.dma_start(out=wo_t, in_=p_wo[of])
                    for tb in range(2):
                        ps = qps.tile([128, 512], F32, tag="oproj")
                        for k in range(8):
                            nc.tensor.matmul(ps, wo_t[:, k, :], Of[:, k, tb * 512:(tb + 1) * 512],
                                             start=(k == 0), stop=(k == 7))
                        xq_t = spool.tile([128, 512], F32R, tag="xqres")
                        nc.sync.dma_start(out=xq_t, in_=xqv[:, of, tb * 512:(tb + 1) * 512])
                        x2 = spool.tile([128, 512], F32R, tag="xf2t")
                        nc.vector.scalar_tensor_tensor(out=x2, in0=ps, scalar=bout[:, of:of + 1],
                                                       in1=xq_t, op0=ALU.add, op1=ALU.add)
                        nc.sync.dma_start(out=xf2v[:, of, tb * 512:(tb + 1) * 512], in_=x2)

        # ---- Phase 5: LayerNorm 2 (from scratch DRAM) ----
        with tc.tile_pool(name="h2p", bufs=1) as h2p:
            H2 = h2p.tile([128, 8, T], F32R, tag="h2")
            with ExitStack() as ph:
                layernorm(ph, d_xf2.ap(), T, ln2g, ln2b, out_tile=H2)

            # ---- Phase 6: FFN + residual + halt ----
            xf2v = d_xf2.ap().rearrange("(ko ki) t -> ki ko t", ki=128)
            hacc = const.tile([128, 8], F32, tag="hacc")
            with ExitStack() as ph:
                wpool = ph.enter_context(tc.tile_pool(name="wffp", bufs=2))
                gpool = ph.enter_context(tc.tile_pool(name="gffp", bufs=1))
                spool = ph.enter_context(tc.tile_pool(name="ffsb", bufs=3))
                fps = ph.enter_context(tc.tile_pool(name="ffps", bufs=2, space="PSUM"))
                for th in range(2):
                    c0 = th * 512
                    gts = []
                    for fb in range(32):
                        w1_t = wpool.tile([128, 8, 128], F32R, tag="w1")
                        nc.sync.dma_start(out=w1_t, in_=p_w1[fb])
                        ps1 = fps.tile([128, 512], F32, tag="ff1")
                        for k in range(8):
                            nc.tensor.matmul(ps1, w1_t[:, k, :], H2[:, k, c0:c0 + 512],
                                             start=(k == 0), stop=(k == 7))
                        g_t = gpool.tile([128, 512], F32R, tag=f"g{fb}")
                        nc.scalar.activation(out=g_t, in_=ps1, func=AF.Gelu,
                                             bias=bff1[:, fb:fb + 1], scale=1.0)
                        gts.append(g_t)
                    for of in range(8):
                        w2_t = wpool.tile([128, 32, 128], F32R, tag="w2")
                        nc.sync.dma_start(out=w2_t, in_=p_w2[of])
                        ps2 = fps.tile([128, 512], F32, tag="ff2")
                        for fb in range(32):
                            nc.tensor.matmul(ps2, w2_t[:, fb, :], gts[fb],
                                             start=(fb == 0), stop=(fb == 31))
                        xf2s = spool.tile([128, 512], F32R, tag="xf2res")
                        nc.sync.dma_start(out=xf2s, in_=xf2v[:, of, c0:c0 + 512])
                        xo = spool.tile([128, 512], F32, tag="xout")
                        nc.vector.scalar_tensor_tensor(out=xo, in0=ps2, scalar=bff2[:, of:of + 1],
                                                       in1=xf2s, op0=ALU.add, op1=ALU.add)
                        nc.sync.dma_start(out=p_xout[of * 128:(of + 1) * 128, c0:c0 + 512], in_=xo)
                        # halt partial: accumulate token-sums of the final x
                        rt = spool.tile([128, 1], F32, tag="hred")
                        nc.vector.reduce_sum(out=rt, in_=xo, axis=mybir.AxisListType.X)
                        if th == 0:
                            nc.vector.tensor_copy(out=hacc[:, of:of + 1], in_=rt)
                        else:
                            nc.vector.tensor_add(out=hacc[:, of:of + 1], in0=hacc[:, of:of + 1], in1=rt)
                hw_t = const.tile([128, 8], F32, tag="hw")
                nc.vector.tensor_mul(out=hw_t, in0=hacc, in1=whalt)
                hredf = const.tile([128, 1], F32, tag="hredf")
                nc.vector.reduce_sum(out=hredf, in_=hw_t, axis=mybir.AxisListType.X)
                nc.sync.dma_start(out=p_halt[:, :], in_=hredf)

    nc.finalize()
    return nc


def _pack_w(w, ncb):
    """[Din, Dout] -> [Dout/128, 128, Din/128, 128] with [cb, ki, ko, m] = w[ko*128+ki, cb*128+m]."""
    din, dout = w.shape
    return np.ascontiguousarray(
        w.reshape(din // 128, 128, dout // 128, 128).transpose(2, 1, 0, 3))


def _make_masks(a_tiles):
    """Additive S^T masks, [NKT, 128 keys, 128 queries], for the first query block of each kt strip."""
    m = np.zeros((NKT, 128, 128), np.float32)
    kk = np.arange(128)[:, None]
    qq = np.arange(128)[None, :]
    for kt in range(NKT):
        aj = a_tiles[kt // 2]
        if aj == kt:
            m[kt] = np.where(kk <= qq, 0.0, MASKVAL)
        elif aj < kt:
            m[kt] = MASKVAL
    return m


def kernel(**inputs):
    global _PROG
    from concourse.bass_utils import run_bass_kernel_spmd

    if _PROG is None:
        _PROG = _build_program()
    nc = _PROG

    x = np.asarray(inputs["x"], np.float32)
    w_qkv = np.asarray(inputs["w_qkv"], np.float32)
    wq, wk, wv = w_qkv[:, :INNER], w_qkv[:, INNER:2 * INNER], w_qkv[:, 2 * INNER:]
    shared = {
        "wq": _pack_w(wq, 8), "wk": _pack_w(wk, 8), "wv": _pack_w(wv, 8),
        "wo": _pack_w(np.asarray(inputs["w_out"], np.float32), 8),
        "w1": _pack_w(np.asarray(inputs["w_ff1"], np.float32), 32),
        "w2": _pack_w(np.asarray(inputs["w_ff2"], np.float32), 8),
        "ln1g": np.ascontiguousarray(np.asarray(inputs["ln1_g"], np.float32).reshape(8, 128).T),
        "ln1b": np.ascontiguousarray(np.asarray(inputs["ln1_b"], np.float32).reshape(8, 128).T),
        "ln2g": np.ascontiguousarray(np.asarray(inputs["ln2_g"], np.float32).reshape(8, 128).T),
        "ln2b": np.ascontiguousarray(np.asarray(inputs["ln2_b"], np.float32).reshape(8, 128).T),
        "bout": np.ascontiguousarray(np.asarray(inputs["b_out"], np.float32).reshape(8, 128).T),
        "bff1": np.ascontiguousarray(np.asarray(inputs["b_ff1"], np.float32).reshape(32, 128).T),
        "bff2": np.ascontiguousarray(np.asarray(inputs["b_ff2"], np.float32).reshape(8, 128).T),
        "whalt": np.ascontiguousarray(np.asarray(inputs["w_halt"], np.float32).reshape(8, 128).T),
    }
    masks = [_make_masks(A_TILES[0]), _make_masks(A_TILES[1])]

    in_maps = []
    for c in range(8):
        b, half = c // 2, c % 2
        tiles = A_TILES[half]
        xb = x[b]                                  # [N, D]
        xq = np.concatenate([xb[t * 128:(t + 1) * 128] for t in tiles], axis=0)  # [T, D]
        in_maps.append({
            "xq": np.ascontiguousarray(xq.T),
            "xkv": np.ascontiguousarray(xb.T),
            "maskt": masks[half],
            **shared,
        })

    res = run_bass_kernel_spmd(nc, in_maps, list(range(8)))
    globals()["LAST_RESULT"] = res
    globals()["LAST_IN_MAPS"] = in_maps

    x_out = np.empty((B, N, D), np.float32)
    hp = np.empty(8, np.float64)
    for c in range(8):
        b, half = c // 2, c % 2
        tiles = A_TILES[half]
        xo = res.results[c]["xout"].T              # [T tokens, D]
        for j, t in enumerate(tiles):
            x_out[b, t * 128:(t + 1) * 128] = xo[j * 128:(j + 1) * 128]
        hp[c] = res.results[c]["hpart"].sum()
    b_halt = np.asarray(inputs["b_halt"], np.float32)
    halt = ((hp[0::2] + hp[1::2]) / N + b_halt[0]).astype(np.float32)
    return x_out, halt
